# revision 6
# baseline (speedup 1.0000x reference)
"""CSAB (cross-set attention block) Trainium2 kernel, v2.

Full inputs in, full outputs out. Data-parallel over batch B=8 across
the 8 NeuronCores, one batch element per core.

Per-core dataflow (all matmuls bf16, fp32 PSUM accumulation), activations
kept feature-major (transposed) so every matmul contracts over the
partition dim:
  Q^T, K^T [D, N]  from lhsT=W chunks, rhs=X^T
  V        [N, D]  token-major, augmented per-head with a ones column
  S^T[k,q] = (K_h^T chunk).T @ Q_h^T  -- two heads of a pair as
             concurrent row-tiled matmuls (partitions 0:64 / 64:128)
  E^T      = exp(S^T / sqrt(D))       -- ScalarE, scale folded in
  o'^T[65,q] = V'_h.T @ E_h^T         -- row 64 = softmax denominator

v2 structural changes vs v1:
  - pair-major weave: per branch, Q/K projections are emitted per-ofc
    (= per head-pair) and interleaved with the attention of earlier
    pairs, so the PE starts QK/exp work ~30us earlier and always has
    ready matmuls while exps trail on the Scalar engine.
  - per-(pair,qh) normalize instead of per-qh batched: the two
    denominator rows are copied (f32) into a [2,512] tile, inverted with
    reciprocal_approx_fast (single custom-DVE op, ~5x faster than
    InstReciprocal), bounced once through DRAM for the zero-stride
    partition broadcast, then normalize+q-residual on GpSimd.  This
    drops the old [8,512] InstReciprocal (4us each) and the double
    DRAM repack, and lets each pair's normalize overlap the next pair's
    AV matmuls.
  - fc is emitted in (out, qh, ofc) chunks of 8 matmuls, popped one per
    av_norm point of the following branch; the final fc(y) chunks are
    ordered qh0-then-qh1 so the last chunks' dependencies (yy qh1
    normalize) resolve while fc(y, qh0) streams.
"""

import math

import numpy as np
import ml_dtypes

import concourse.bass as bass
import concourse.mybir as mybir
import concourse.tile as tile
from concourse.bass_utils import run_bass_kernel_spmd

B, N, D, H = 8, 1024, 512, 8
DH = D // H          # 64
P = 128
KC = D // P          # 4 feature chunks
QH = N // 512        # 2 q halves
KT = N // P          # 8 k tiles
NPAIR = H // 2       # 4 head pairs
SCALE = 1.0 / math.sqrt(D)

F32 = mybir.dt.float32
BF16 = mybir.dt.bfloat16
AF = mybir.ActivationFunctionType
ALU = mybir.AluOpType

_BRANCHES = [("xx", "x", "x"), ("xy", "x", "y"), ("yx", "y", "x"), ("yy", "y", "y")]

LAST_RESULT = None
_CACHED_NC = None


def _split_excess_waits(nc):
    """The walrus build in this container accepts at most one sync-wait
    per instruction (two for EventSemaphore). Tile's scheduler emits
    several on some instructions. Hoist the overflow onto same-engine
    NoOps inserted immediately before the instruction — the engine
    blocks at the nops instead, so the wait point in the instruction
    stream is unchanged."""
    cap_of = {"InstEventSemaphore": 2}

    def cap_for(inst):
        if getattr(inst, "is_scalar_tensor_tensor", False):
            return 0   # the STT ISA struct has no sync-wait slot
        return cap_of.get(type(inst).__name__, 1)
    # Pass 1: strip overflow waits off each instruction, remember them.
    plans = []
    for f in nc.m.functions:
        for bb in f.blocks:
            plan = []
            for inst in list(bb.instructions):
                si = getattr(inst, "sync_info", None)
                waits = list(si.on_wait) if si and si.on_wait else []
                cap = cap_for(inst)
                if len(waits) > cap:
                    cut = len(waits) - cap
                    plan.append((inst, waits[:cut]))
                    si.on_wait = waits[cut:]
            plans.append((bb, plan))
    # Pass 2: create the carrier nops. The engine builder appends them to
    # whatever block is current — they are stripped by name in pass 3 and
    # re-inserted at their proper position.
    nop_map = {}
    created = set()
    for bb, plan in plans:
        for inst, extra in plan:
            nops = []
            for w in extra:
                ni = nc.engines[inst.engine].nop(hint="waitsplit")
                ni.ins.sync_info = mybir.SyncInfo(on_wait=[w], on_update=[])
                nops.append(ni.ins)
                created.add(ni.ins.name)
            nop_map[inst.name] = nops
    # Pass 3: rebuild each block: drop stray auto-appended copies, insert
    # each nop chain immediately before its instruction.
    for bb, plan in plans:
        live = [i for i in bb.instructions if i.name not in created]
        new = []
        for inst in live:
            new.extend(nop_map.get(inst.name, ()))
            new.append(inst)
        bb.instructions = new


def _build_nc():
    nc = bass.Bass()

    # ---- DRAM I/O (per core) -------------------------------------------
    xt_bf = nc.dram_tensor("xt_bf", [D, N], BF16, kind="ExternalInput")
    yt_bf = nc.dram_tensor("yt_bf", [D, N], BF16, kind="ExternalInput")
    xt_f32 = nc.dram_tensor("xt_f32", [D, N], F32, kind="ExternalInput")
    yt_f32 = nc.dram_tensor("yt_f32", [D, N], F32, kind="ExternalInput")
    wdr = {}
    for bn, _, _ in _BRANCHES:
        for t in ("q", "k", "v"):
            wdr[f"w{t}_{bn}"] = nc.dram_tensor(f"w{t}_{bn}", [D, D], BF16,
                                               kind="ExternalInput")
            wdr[f"b{t}_{bn}"] = nc.dram_tensor(f"b{t}_{bn}", [D], F32,
                                               kind="ExternalInput")
    wfc_x = nc.dram_tensor("wfc_x", [2 * D, D], BF16, kind="ExternalInput")
    wfc_y = nc.dram_tensor("wfc_y", [2 * D, D], BF16, kind="ExternalInput")
    bfc_x = nc.dram_tensor("bfc_x", [D], F32, kind="ExternalInput")
    bfc_y = nc.dram_tensor("bfc_y", [D], F32, kind="ExternalInput")
    out_x_t = nc.dram_tensor("out_x_t", [D, N], F32, kind="ExternalOutput")
    out_y_t = nc.dram_tensor("out_y_t", [D, N], F32, kind="ExternalOutput")

    with tile.TileContext(nc) as tc, \
         tc.tile_pool(name="const", bufs=1) as const_pool, \
         tc.tile_pool(name="acts", bufs=1) as acts_pool, \
         tc.tile_pool(name="wbr", bufs=2) as wbr_pool, \
         tc.tile_pool(name="qkv_q", bufs=2) as q_pool, \
         tc.tile_pool(name="qkv_kv", bufs=2) as kv_pool, \
         tc.tile_pool(name="epool", bufs=2) as e_pool, \
         tc.tile_pool(name="norm", bufs=4) as norm_pool, \
         tc.tile_pool(name="norm2", bufs=2) as norm2_pool, \
         tc.tile_pool(name="attn", bufs=3) as attn_pool, \
         tc.tile_pool(name="stream", bufs=2) as stream_pool, \
         tc.tile_pool(name="dbounce", bufs=3, space="DRAM") as dram_pool, \
         tc.tile_pool(name="qk_ps", bufs=2, space="PSUM") as qk_ps_pool, \
         tc.tile_pool(name="misc_ps", bufs=2, space="PSUM") as misc_ps_pool, \
         tc.tile_pool(name="proj_ps", bufs=2, space="PSUM") as proj_ps_pool:

        # ---- resident activations --------------------------------------
        xt_sb = acts_pool.tile([P, KC, N], BF16, tag="xt_sb")
        yt_sb = acts_pool.tile([P, KC, N], BF16, tag="yt_sb")
        for kc in range(KC):
            nc.sync.dma_start(
                xt_sb[:, kc, :],
                xt_bf.rearrange("(o p) n -> p o n", p=P)[:, kc, :])
        for kc in range(KC):
            nc.sync.dma_start(
                yt_sb[:, kc, :],
                yt_bf.rearrange("(o p) n -> p o n", p=P)[:, kc, :])
        act_sb = {"x": xt_sb, "y": yt_sb}
        act_res_dram = {"x": xt_f32, "y": yt_f32}

        wfc_sb = {}
        bfc_sb = {}
        for nm, wd, bd in (("x", wfc_x, bfc_x), ("y", wfc_y, bfc_y)):
            w = const_pool.tile([P, 2 * KC, D], BF16, tag=f"wfc_{nm}")
            nc.sync.dma_start(w[:], wd.rearrange("(o p) f -> p o f", p=P))
            bt = const_pool.tile([P, KC], F32, tag=f"bfc_{nm}")
            nc.sync.dma_start(bt[:], bd.rearrange("(o p) -> p o", p=P))
            wfc_sb[nm] = w
            bfc_sb[nm] = bt

        attn_sb_of = {}   # branch name -> O^T (= attn + Q residual) tile

        # fc work queue: items are (out_name, qh, ofc); one item = 8 matmuls
        pending_fc = []

        def fc_chunk(out_name, qh, ofc):
            w_sb = wfc_sb[out_name]
            b_sb = bfc_sb[out_name]
            out_dram = out_x_t if out_name == "x" else out_y_t
            res_dram = act_res_dram[out_name]
            br0, br1 = (("xx", "xy") if out_name == "x" else ("yx", "yy"))
            srcs = [(attn_sb_of[br0], 0), (attn_sb_of[br1], KC)]
            # fc reuses the qk pool's banks: first 512 cols of [P,1024].
            ps_t = qk_ps_pool.tile([P, 1024], F32, tag="qk")
            ps = ps_t[:, 0:512]
            step = 0
            for src_sb, wbase in srcs:
                for kc in range(KC):
                    nc.tensor.matmul(
                        ps,
                        lhsT=w_sb[:, wbase + kc, ofc * P:(ofc + 1) * P],
                        rhs=src_sb[:, kc, qh * 512:(qh + 1) * 512],
                        start=(step == 0), stop=(step == 2 * KC - 1),
                    )
                    step += 1
            xres = stream_pool.tile([P, 512], F32, tag="xres")
            nc.sync.dma_start(
                xres[:],
                res_dram[ofc * P:(ofc + 1) * P, qh * 512:(qh + 1) * 512],
            )
            outt = stream_pool.tile([P, 512], F32, tag="outt")
            nc.vector.tensor_scalar(
                outt[:], ps,
                b_sb[:, ofc:ofc + 1], 0.0,
                ALU.add, ALU.max,
            )
            nc.vector.tensor_tensor(outt[:], outt[:], xres[:], ALU.add)
            nc.sync.dma_start(
                out_dram[ofc * P:(ofc + 1) * P, qh * 512:(qh + 1) * 512],
                outt[:],
            )

        def pop_fc():
            if pending_fc:
                fc_chunk(*pending_fc.pop(0))

        # ---- branches ---------------------------------------------------
        for bname, qsrc, kvsrc in _BRANCHES:
            # branch weights / biases
            w_sb = {}
            for t in ("q", "k", "v"):
                w = wbr_pool.tile([P, KC, D], BF16, tag=f"w{t}")
                nc.sync.dma_start(
                    w[:], wdr[f"w{t}_{bname}"].rearrange("(o p) f -> p o f", p=P))
                w_sb[t] = w
            bq_sb = wbr_pool.tile([P, KC], F32, tag="bq")
            nc.sync.dma_start(
                bq_sb[:], wdr[f"bq_{bname}"].rearrange("(o p) -> p o", p=P))
            bk_sb = wbr_pool.tile([P, KC], F32, tag="bk")
            nc.sync.dma_start(
                bk_sb[:], wdr[f"bk_{bname}"].rearrange("(o p) -> p o", p=P))
            # V bias broadcast across all 128 partitions (tokens)
            bvb_sb = wbr_pool.tile([P, D], BF16, tag="bvb")
            nc.gpsimd.dma_start(
                out=bvb_sb[:],
                in_=wdr[f"bv_{bname}"][:].partition_broadcast(P),
            )

            qt_sb = q_pool.tile([P, KC, N], BF16, tag="qt")
            kt_sb = kv_pool.tile([P, KC, N], BF16, tag="kt")
            vaug_sb = kv_pool.tile([P, KT, H * (DH + 1)], BF16, tag="vaug")
            attn_sb = attn_pool.tile([P, KC, N], BF16, tag="attn")
            attn_sb_of[bname] = attn_sb

            src_q = act_sb[qsrc]
            src_kv = act_sb[kvsrc]

            def proj_qk(ofc):
                # Q^T and K^T feature chunk ofc (= head pair ofc)
                for w, b_t, out_sb in ((w_sb["q"], bq_sb, qt_sb),
                                       (w_sb["k"], bk_sb, kt_sb)):
                    src = src_q if out_sb is qt_sb else src_kv
                    for qh in range(QH):
                        ps = proj_ps_pool.tile([P, 512], F32, tag="proj")
                        for kc in range(KC):
                            nc.tensor.matmul(
                                ps[:],
                                lhsT=w[:, kc, ofc * P:(ofc + 1) * P],
                                rhs=src[:, kc, qh * 512:(qh + 1) * 512],
                                start=(kc == 0), stop=(kc == KC - 1),
                            )
                        nc.vector.tensor_scalar_add(
                            out_sb[:, ofc, qh * 512:(qh + 1) * 512],
                            ps[:],
                            b_t[:, ofc:ofc + 1],
                        )

            def proj_v():
                # V[t, f] token-major, scattered into per-head 65-col blocks
                for tt in range(KT):
                    ps = proj_ps_pool.tile([P, 512], F32, tag="proj")
                    for kc in range(KC):
                        nc.tensor.matmul(
                            ps[:],
                            lhsT=src_kv[:, kc, tt * P:(tt + 1) * P],
                            rhs=w_sb["v"][:, kc, :],
                            start=(kc == 0), stop=(kc == KC - 1),
                        )
                    dst = vaug_sb[:, tt, :].rearrange("p (h c) -> p h c", c=DH + 1)
                    nc.vector.tensor_tensor(
                        dst[:, :, :DH],
                        ps[:].rearrange("p (h c) -> p h c", c=DH),
                        bvb_sb[:].rearrange("p (h c) -> p h c", c=DH),
                        ALU.add,
                    )
                # ones columns of V' (col 64 of each 65-wide head block)
                nc.gpsimd.memset(
                    vaug_sb[:].rearrange(
                        "p t (h c) -> p t h c", c=DH + 1)[:, :, :, DH:],
                    1.0,
                )

            e_map = {}

            def qk_exp(pair, qh):
                qcols = slice(qh * 512, (qh + 1) * 512)
                e_sb = e_pool.tile([P, KT, 2, 512], BF16, tag="e")
                e_map[(pair, qh)] = e_sb
                for kt in range(KT):
                    ps = qk_ps_pool.tile([P, 1024], F32, tag="qk")
                    # head A on array rows 0:64, head B on rows 64:128
                    nc.tensor.matmul(
                        ps[:, 0:512],
                        lhsT=kt_sb[0:DH, pair, kt * P:(kt + 1) * P],
                        rhs=qt_sb[0:DH, pair, qcols],
                        start=True, stop=True,
                    )
                    nc.tensor.matmul(
                        ps[:, 512:1024],
                        lhsT=kt_sb[DH:P, pair, kt * P:(kt + 1) * P],
                        rhs=qt_sb[DH:P, pair, qcols],
                        start=True, stop=True,
                    )
                    nc.scalar.activation(
                        e_sb[:, kt, :, :], ps[:],
                        AF.Exp, scale=SCALE,
                    )

            def av_norm(pair, qh):
                qcols = slice(qh * 512, (qh + 1) * 512)
                e_sb = e_map.pop((pair, qh))
                obuf = norm_pool.tile([P, 512], BF16, tag="obuf")
                # engine writes must be 32-aligned in partitions, so both
                # denominator rows stage into partition 0's free dim
                dp2 = norm2_pool.tile([1, 2, 512], F32, tag="dp2")
                for hl in range(2):
                    h = 2 * pair + hl
                    rows = slice(hl * DH, (hl + 1) * DH)
                    ps_av = misc_ps_pool.tile([P, 512], F32, tag="misc")
                    for kt in range(KT):
                        nc.tensor.matmul(
                            ps_av[:DH + 1, :],
                            lhsT=vaug_sb[:, kt, h * (DH + 1):(h + 1) * (DH + 1)],
                            rhs=e_sb[:, kt, hl, :],
                            start=(kt == 0), stop=(kt == KT - 1),
                        )
                    # drain PSUM immediately: o' rows to SBUF bf16, the
                    # denominator row (f32) into the pair staging tile
                    nc.vector.tensor_copy(obuf[rows, :], ps_av[:DH, :])
                    nc.vector.tensor_copy(
                        dp2[0:1, hl, :], ps_av[DH:DH + 1, :])
                # reciprocal of the two denominator rows, then one DRAM hop
                # for the zero-stride partition broadcast of each head's row.
                rp2 = norm2_pool.tile([1, 2, 512], F32, tag="rp2")
                nc.vector.reciprocal(rp2[:], dp2[:])
                rb_d = dram_pool.tile([2, 512], F32, tag="rbd")
                nc.sync.dma_start(rb_d[:], rp2[0:1, :, :])
                rbc = norm_pool.tile([P, 512], F32, tag="rbc")
                for hl in range(2):
                    rows = slice(hl * DH, (hl + 1) * DH)
                    nc.gpsimd.dma_start(
                        out=rbc[rows, :],
                        in_=rb_d[hl, :].partition_broadcast(DH))
                # both heads stacked in obuf/rbc rows: normalize +
                # q-residual as two [128,512] ops on GpSimd
                nc.gpsimd.tensor_tensor(
                    attn_sb[:, pair, qcols], obuf[:, :], rbc[:, :], ALU.mult)
                nc.gpsimd.tensor_tensor(
                    attn_sb[:, pair, qcols],
                    attn_sb[:, pair, qcols],
                    qt_sb[:, pair, qcols], ALU.add)
                pop_fc()

            # ---- branch emission weave ----------------------------------
            proj_qk(0)
            qk_exp(0, 0)
            proj_qk(1)
            qk_exp(0, 1)
            proj_v()
            av_norm(0, 0)
            qk_exp(1, 0)
            av_norm(0, 1)
            proj_qk(2)
            qk_exp(1, 1)
            av_norm(1, 0)
            qk_exp(2, 0)
            av_norm(1, 1)
            proj_qk(3)
            qk_exp(2, 1)
            av_norm(2, 0)
            qk_exp(3, 0)
            av_norm(2, 1)
            qk_exp(3, 1)
            av_norm(3, 0)
            av_norm(3, 1)

            if bname == "xy":
                pending_fc.extend(
                    [("x", 0, ofc) for ofc in range(KC)]
                    + [("x", 1, ofc) for ofc in range(KC)])
            elif bname == "yy":
                # tail: fc(y, qh0) streams while yy qh1's last normalize
                # chains resolve, then fc(y, qh1).
                for ofc in range(KC):
                    fc_chunk("y", 0, ofc)
                for ofc in range(KC):
                    fc_chunk("y", 1, ofc)

    _split_excess_waits(nc)
    return nc


def _get_nc():
    global _CACHED_NC
    if _CACHED_NC is None:
        _CACHED_NC = _build_nc()
    return _CACHED_NC


def kernel(**inputs):
    global LAST_RESULT
    nc = _get_nc()

    X = np.asarray(inputs["X"], np.float32)
    Y = np.asarray(inputs["Y"], np.float32)

    def bf(a):
        return np.ascontiguousarray(a).astype(ml_dtypes.bfloat16)

    shared = {}
    for bn, _, _ in _BRANCHES:
        for t in ("q", "k", "v"):
            shared[f"w{t}_{bn}"] = bf(inputs[f"W_{t}_{bn}"])
            shared[f"b{t}_{bn}"] = np.asarray(inputs[f"b_{t}_{bn}"], np.float32)
    shared["wfc_x"] = bf(inputs["W_X"])
    shared["wfc_y"] = bf(inputs["W_Y"])
    shared["bfc_x"] = np.asarray(inputs["b_X"], np.float32)
    shared["bfc_y"] = np.asarray(inputs["b_Y"], np.float32)

    in_maps = []
    for b in range(B):
        xt = np.ascontiguousarray(X[b].T)
        yt = np.ascontiguousarray(Y[b].T)
        m = dict(shared)
        m["xt_bf"] = xt.astype(ml_dtypes.bfloat16)
        m["yt_bf"] = yt.astype(ml_dtypes.bfloat16)
        m["xt_f32"] = xt
        m["yt_f32"] = yt
        in_maps.append(m)

    res = run_bass_kernel_spmd(nc, in_maps, list(range(B)))
    LAST_RESULT = res

    out_x = np.stack([res.results[b]["out_x_t"].T for b in range(B)])
    out_y = np.stack([res.results[b]["out_y_t"].T for b in range(B)])
    return out_x.astype(np.float32), out_y.astype(np.float32)


# revision 13
# speedup vs baseline: 1.3713x; 1.3713x over previous
"""CSAB (cross-set attention block) Trainium2 kernel, v2.

Full inputs in, full outputs out. Data-parallel over batch B=8 across
the 8 NeuronCores, one batch element per core.

Per-core dataflow (all matmuls bf16, fp32 PSUM accumulation), activations
kept feature-major (transposed) so every matmul contracts over the
partition dim:
  Q^T, K^T [D, N]  from lhsT=W chunks, rhs=X^T
  V        [N, D]  token-major, augmented per-head with a ones column
  S^T[k,q] = (K_h^T chunk).T @ Q_h^T  -- two heads of a pair as
             concurrent row-tiled matmuls (partitions 0:64 / 64:128)
  E^T      = exp(S^T / sqrt(D))       -- ScalarE, scale folded in
  o'^T[65,q] = V'_h.T @ E_h^T         -- row 64 = softmax denominator

v2 structural changes vs v1:
  - pair-major weave: per branch, Q/K projections are emitted per-ofc
    (= per head-pair) and interleaved with the attention of earlier
    pairs, so the PE starts QK/exp work ~30us earlier and always has
    ready matmuls while exps trail on the Scalar engine.
  - per-(pair,qh) normalize instead of per-qh batched: the two
    denominator rows are copied (f32) into a [2,512] tile, inverted with
    reciprocal_approx_fast (single custom-DVE op, ~5x faster than
    InstReciprocal), bounced once through DRAM for the zero-stride
    partition broadcast, then normalize+q-residual on GpSimd.  This
    drops the old [8,512] InstReciprocal (4us each) and the double
    DRAM repack, and lets each pair's normalize overlap the next pair's
    AV matmuls.
  - fc is emitted in (out, qh, ofc) chunks of 8 matmuls, popped one per
    av_norm point of the following branch; the final fc(y) chunks are
    ordered qh0-then-qh1 so the last chunks' dependencies (yy qh1
    normalize) resolve while fc(y, qh0) streams.
"""

import math

import numpy as np
import ml_dtypes

import concourse.bass as bass
import concourse.mybir as mybir
import concourse.tile as tile
from concourse.bass_utils import run_bass_kernel_spmd

B, N, D, H = 8, 1024, 512, 8
DH = D // H          # 64
P = 128
KC = D // P          # 4 feature chunks
QH = N // 512        # 2 q halves
KT = N // P          # 8 k tiles
NPAIR = H // 2       # 4 head pairs
SCALE = 1.0 / math.sqrt(D)

F32 = mybir.dt.float32
BF16 = mybir.dt.bfloat16
AF = mybir.ActivationFunctionType
ALU = mybir.AluOpType

_BRANCHES = [("xx", "x", "x"), ("xy", "x", "y"), ("yx", "y", "x"), ("yy", "y", "y")]

LAST_RESULT = None
_CACHED_NC = None


def _split_excess_waits(nc):
    """The walrus build in this container accepts at most one sync-wait
    per instruction (two for EventSemaphore). Tile's scheduler emits
    several on some instructions. Hoist the overflow onto same-engine
    NoOps inserted immediately before the instruction — the engine
    blocks at the nops instead, so the wait point in the instruction
    stream is unchanged."""
    cap_of = {"InstEventSemaphore": 2}

    def cap_for(inst):
        if getattr(inst, "is_scalar_tensor_tensor", False):
            return 0   # the STT ISA struct has no sync-wait slot
        return cap_of.get(type(inst).__name__, 1)
    # Pass 1: strip overflow waits off each instruction, remember them.
    plans = []
    for f in nc.m.functions:
        for bb in f.blocks:
            plan = []
            for inst in list(bb.instructions):
                si = getattr(inst, "sync_info", None)
                waits = list(si.on_wait) if si and si.on_wait else []
                cap = cap_for(inst)
                if len(waits) > cap:
                    cut = len(waits) - cap
                    plan.append((inst, waits[:cut]))
                    si.on_wait = waits[cut:]
            plans.append((bb, plan))
    # Pass 2: create the carrier nops. The engine builder appends them to
    # whatever block is current — they are stripped by name in pass 3 and
    # re-inserted at their proper position.
    nop_map = {}
    created = set()
    for bb, plan in plans:
        for inst, extra in plan:
            nops = []
            for w in extra:
                ni = nc.engines[inst.engine].nop(hint="waitsplit")
                ni.ins.sync_info = mybir.SyncInfo(on_wait=[w], on_update=[])
                nops.append(ni.ins)
                created.add(ni.ins.name)
            nop_map[inst.name] = nops
    # Pass 3: rebuild each block: drop stray auto-appended copies, insert
    # each nop chain immediately before its instruction.
    for bb, plan in plans:
        live = [i for i in bb.instructions if i.name not in created]
        new = []
        for inst in live:
            new.extend(nop_map.get(inst.name, ()))
            new.append(inst)
        bb.instructions = new


def _build_nc():
    nc = bass.Bass()

    # ---- DRAM I/O (per core) -------------------------------------------
    xt_bf = nc.dram_tensor("xt_bf", [D, N], BF16, kind="ExternalInput")
    yt_bf = nc.dram_tensor("yt_bf", [D, N], BF16, kind="ExternalInput")
    xt_f32 = nc.dram_tensor("xt_f32", [D, N], F32, kind="ExternalInput")
    yt_f32 = nc.dram_tensor("yt_f32", [D, N], F32, kind="ExternalInput")
    wdr = {}
    for bn, _, _ in _BRANCHES:
        for t in ("q", "k", "v"):
            wdr[f"w{t}_{bn}"] = nc.dram_tensor(f"w{t}_{bn}", [D, D], BF16,
                                               kind="ExternalInput")
            wdr[f"b{t}_{bn}"] = nc.dram_tensor(f"b{t}_{bn}", [D], F32,
                                               kind="ExternalInput")
    wfc_x = nc.dram_tensor("wfc_x", [2 * D, D], BF16, kind="ExternalInput")
    wfc_y = nc.dram_tensor("wfc_y", [2 * D, D], BF16, kind="ExternalInput")
    bfc_x = nc.dram_tensor("bfc_x", [D], F32, kind="ExternalInput")
    bfc_y = nc.dram_tensor("bfc_y", [D], F32, kind="ExternalInput")
    out_x_t = nc.dram_tensor("out_x_t", [D, N], F32, kind="ExternalOutput")
    out_y_t = nc.dram_tensor("out_y_t", [D, N], F32, kind="ExternalOutput")

    with tile.TileContext(nc) as tc, \
         tc.tile_pool(name="const", bufs=1) as const_pool, \
         tc.tile_pool(name="acts", bufs=1) as acts_pool, \
         tc.tile_pool(name="wbr", bufs=2) as wbr_pool, \
         tc.tile_pool(name="qkv_q", bufs=2) as q_pool, \
         tc.tile_pool(name="qkv_kv", bufs=2) as kv_pool, \
         tc.tile_pool(name="epool", bufs=2) as e_pool, \
         tc.tile_pool(name="norm", bufs=4) as norm_pool, \
         tc.tile_pool(name="norm2", bufs=2) as norm2_pool, \
         tc.tile_pool(name="attn", bufs=3) as attn_pool, \
         tc.tile_pool(name="stream", bufs=2) as stream_pool, \
         tc.tile_pool(name="dbounce", bufs=3, space="DRAM") as dram_pool, \
         tc.tile_pool(name="qk_ps", bufs=2, space="PSUM") as qk_ps_pool, \
         tc.tile_pool(name="misc_ps", bufs=2, space="PSUM") as misc_ps_pool, \
         tc.tile_pool(name="proj_ps", bufs=2, space="PSUM") as proj_ps_pool:

        # ---- resident activations --------------------------------------
        xt_sb = acts_pool.tile([P, KC, N], BF16, tag="xt_sb")
        yt_sb = acts_pool.tile([P, KC, N], BF16, tag="yt_sb")
        for kc in range(KC):
            nc.sync.dma_start(
                xt_sb[:, kc, :],
                xt_bf.rearrange("(o p) n -> p o n", p=P)[:, kc, :])
        for kc in range(KC):
            nc.sync.dma_start(
                yt_sb[:, kc, :],
                yt_bf.rearrange("(o p) n -> p o n", p=P)[:, kc, :])
        act_sb = {"x": xt_sb, "y": yt_sb}
        act_res_dram = {"x": xt_f32, "y": yt_f32}

        wfc_sb = {}
        bfc_sb = {}
        for nm, wd, bd in (("x", wfc_x, bfc_x), ("y", wfc_y, bfc_y)):
            w = const_pool.tile([P, 2 * KC, D], BF16, tag=f"wfc_{nm}")
            nc.sync.dma_start(w[:], wd.rearrange("(o p) f -> p o f", p=P))
            bt = const_pool.tile([P, KC], F32, tag=f"bfc_{nm}")
            nc.sync.dma_start(bt[:], bd.rearrange("(o p) -> p o", p=P))
            wfc_sb[nm] = w
            bfc_sb[nm] = bt

        attn_sb_of = {}   # branch name -> O^T (= attn + Q residual) tile

        # fc work queue: items are (out_name, qh, ofc); one item = 8 matmuls
        pending_fc = []

        def fc_chunk(out_name, qh, ofc):
            w_sb = wfc_sb[out_name]
            b_sb = bfc_sb[out_name]
            out_dram = out_x_t if out_name == "x" else out_y_t
            res_dram = act_res_dram[out_name]
            br0, br1 = (("xx", "xy") if out_name == "x" else ("yx", "yy"))
            srcs = [(attn_sb_of[br0], 0), (attn_sb_of[br1], KC)]
            # fc reuses the qk pool's banks: first 512 cols of [P,1024].
            ps_t = qk_ps_pool.tile([P, 1024], F32, tag="qk")
            ps = ps_t[:, 0:512]
            step = 0
            for src_sb, wbase in srcs:
                for kc in range(KC):
                    nc.tensor.matmul(
                        ps,
                        lhsT=w_sb[:, wbase + kc, ofc * P:(ofc + 1) * P],
                        rhs=src_sb[:, kc, qh * 512:(qh + 1) * 512],
                        start=(step == 0), stop=(step == 2 * KC - 1),
                    )
                    step += 1
            xres = stream_pool.tile([P, 512], F32, tag="xres")
            nc.sync.dma_start(
                xres[:],
                res_dram[ofc * P:(ofc + 1) * P, qh * 512:(qh + 1) * 512],
            )
            outt = stream_pool.tile([P, 512], F32, tag="outt")
            nc.vector.tensor_scalar(
                outt[:], ps,
                b_sb[:, ofc:ofc + 1], 0.0,
                ALU.add, ALU.max,
            )
            nc.vector.tensor_tensor(outt[:], outt[:], xres[:], ALU.add)
            nc.sync.dma_start(
                out_dram[ofc * P:(ofc + 1) * P, qh * 512:(qh + 1) * 512],
                outt[:],
            )

        def pop_fc():
            if pending_fc:
                fc_chunk(*pending_fc.pop(0))

        # ---- branches ---------------------------------------------------
        for bname, qsrc, kvsrc in _BRANCHES:
            # branch weights / biases
            w_sb = {}
            for t in ("q", "k", "v"):
                w = wbr_pool.tile([P, KC, D], BF16, tag=f"w{t}")
                nc.sync.dma_start(
                    w[:], wdr[f"w{t}_{bname}"].rearrange("(o p) f -> p o f", p=P))
                w_sb[t] = w
            bq_sb = wbr_pool.tile([P, KC], F32, tag="bq")
            nc.sync.dma_start(
                bq_sb[:], wdr[f"bq_{bname}"].rearrange("(o p) -> p o", p=P))
            bk_sb = wbr_pool.tile([P, KC], F32, tag="bk")
            nc.sync.dma_start(
                bk_sb[:], wdr[f"bk_{bname}"].rearrange("(o p) -> p o", p=P))
            # V bias broadcast across all 128 partitions (tokens)
            bvb_sb = wbr_pool.tile([P, D], BF16, tag="bvb")
            nc.gpsimd.dma_start(
                out=bvb_sb[:],
                in_=wdr[f"bv_{bname}"][:].partition_broadcast(P),
            )

            qt_sb = q_pool.tile([P, KC, N], BF16, tag="qt")
            kt_sb = kv_pool.tile([P, KC, N], BF16, tag="kt")
            vaug_sb = kv_pool.tile([P, KT, H * (DH + 1)], BF16, tag="vaug")
            attn_sb = attn_pool.tile([P, KC, N], BF16, tag="attn")
            attn_sb_of[bname] = attn_sb

            src_q = act_sb[qsrc]
            src_kv = act_sb[kvsrc]

            def proj_qk(ofc):
                # Q^T and K^T feature chunk ofc (= head pair ofc)
                for w, b_t, out_sb in ((w_sb["q"], bq_sb, qt_sb),
                                       (w_sb["k"], bk_sb, kt_sb)):
                    src = src_q if out_sb is qt_sb else src_kv
                    for qh in range(QH):
                        ps = proj_ps_pool.tile([P, 512], F32, tag="proj")
                        for kc in range(KC):
                            nc.tensor.matmul(
                                ps[:],
                                lhsT=w[:, kc, ofc * P:(ofc + 1) * P],
                                rhs=src[:, kc, qh * 512:(qh + 1) * 512],
                                start=(kc == 0), stop=(kc == KC - 1),
                            )
                        nc.vector.tensor_scalar_add(
                            out_sb[:, ofc, qh * 512:(qh + 1) * 512],
                            ps[:],
                            b_t[:, ofc:ofc + 1],
                        )

            def proj_v():
                # V[t, f] token-major, scattered into per-head 65-col blocks
                for tt in range(KT):
                    ps = proj_ps_pool.tile([P, 512], F32, tag="proj")
                    for kc in range(KC):
                        nc.tensor.matmul(
                            ps[:],
                            lhsT=src_kv[:, kc, tt * P:(tt + 1) * P],
                            rhs=w_sb["v"][:, kc, :],
                            start=(kc == 0), stop=(kc == KC - 1),
                        )
                    dst = vaug_sb[:, tt, :].rearrange("p (h c) -> p h c", c=DH + 1)
                    nc.vector.tensor_tensor(
                        dst[:, :, :DH],
                        ps[:].rearrange("p (h c) -> p h c", c=DH),
                        bvb_sb[:].rearrange("p (h c) -> p h c", c=DH),
                        ALU.add,
                    )
                # ones columns of V' (col 64 of each 65-wide head block)
                nc.gpsimd.memset(
                    vaug_sb[:].rearrange(
                        "p t (h c) -> p t h c", c=DH + 1)[:, :, :, DH:],
                    1.0,
                )

            e_map = {}

            def qk_exp(pair, qh):
                qcols = slice(qh * 512, (qh + 1) * 512)
                e_sb = e_pool.tile([P, KT, 2, 512], BF16, tag="e")
                e_map[(pair, qh)] = e_sb
                for kt in range(KT):
                    ps = qk_ps_pool.tile([P, 1024], F32, tag="qk")
                    # head A on array rows 0:64, head B on rows 64:128
                    nc.tensor.matmul(
                        ps[:, 0:512],
                        lhsT=kt_sb[0:DH, pair, kt * P:(kt + 1) * P],
                        rhs=qt_sb[0:DH, pair, qcols],
                        start=True, stop=True,
                    )
                    nc.tensor.matmul(
                        ps[:, 512:1024],
                        lhsT=kt_sb[DH:P, pair, kt * P:(kt + 1) * P],
                        rhs=qt_sb[DH:P, pair, qcols],
                        start=True, stop=True,
                    )
                    nc.scalar.activation(
                        e_sb[:, kt, :, :], ps[:],
                        AF.Exp, scale=SCALE,
                    )

            def av_norm(pair, qh):
                qcols = slice(qh * 512, (qh + 1) * 512)
                e_sb = e_map.pop((pair, qh))
                obuf = norm_pool.tile([P, 512], BF16, tag="obuf")
                # engine writes must be 32-aligned in partitions, so both
                # denominator rows stage into partition 0's free dim
                dp2 = norm2_pool.tile([1, 2, 512], F32, tag="dp2")
                for hl in range(2):
                    h = 2 * pair + hl
                    rows = slice(hl * DH, (hl + 1) * DH)
                    ps_av = misc_ps_pool.tile([P, 512], F32, tag="misc")
                    for kt in range(KT):
                        nc.tensor.matmul(
                            ps_av[:DH + 1, :],
                            lhsT=vaug_sb[:, kt, h * (DH + 1):(h + 1) * (DH + 1)],
                            rhs=e_sb[:, kt, hl, :],
                            start=(kt == 0), stop=(kt == KT - 1),
                        )
                    # drain PSUM immediately: o' rows to SBUF bf16, the
                    # denominator row (f32) into the pair staging tile
                    nc.vector.tensor_copy(obuf[rows, :], ps_av[:DH, :])
                    nc.vector.tensor_copy(
                        dp2[0:1, hl, :], ps_av[DH:DH + 1, :])
                # InstReciprocal costs ~8 cycles per FREE element (iterated
                # divide), partition-parallel — so bounce the 1024 d values
                # through DRAM into a [32,32] partition-major shape first:
                # free size 32 makes the reciprocal ~0.5us instead of ~6.5us.
                d_d = dram_pool.tile([2, 512], F32, tag="dd")
                nc.sync.dma_start(d_d[:], dp2[0:1, :, :])
                dp32 = norm2_pool.tile([32, 32], F32, tag="dp32")
                nc.sync.dma_start(
                    dp32[:], d_d.rearrange("a (c d) -> (a c) d", d=32))
                rp32 = norm2_pool.tile([32, 32], F32, tag="rp32")
                nc.vector.reciprocal(rp32[:], dp32[:])
                rb_d = dram_pool.tile([2, 512], F32, tag="rbd")
                nc.sync.dma_start(
                    rb_d.rearrange("a (c d) -> (a c) d", d=32), rp32[:])
                rbc = norm_pool.tile([P, 512], F32, tag="rbc")
                for hl in range(2):
                    rows = slice(hl * DH, (hl + 1) * DH)
                    nc.gpsimd.dma_start(
                        out=rbc[rows, :],
                        in_=rb_d[hl, :].partition_broadcast(DH))
                # both heads stacked in obuf/rbc rows: normalize +
                # q-residual as two [128,512] ops on GpSimd
                nc.gpsimd.tensor_tensor(
                    attn_sb[:, pair, qcols], obuf[:, :], rbc[:, :], ALU.mult)
                nc.gpsimd.tensor_tensor(
                    attn_sb[:, pair, qcols],
                    attn_sb[:, pair, qcols],
                    qt_sb[:, pair, qcols], ALU.add)
                pop_fc()

            # ---- branch emission weave ----------------------------------
            proj_qk(0)
            qk_exp(0, 0)
            proj_qk(1)
            qk_exp(0, 1)
            proj_v()
            av_norm(0, 0)
            qk_exp(1, 0)
            av_norm(0, 1)
            proj_qk(2)
            qk_exp(1, 1)
            av_norm(1, 0)
            qk_exp(2, 0)
            av_norm(1, 1)
            proj_qk(3)
            qk_exp(2, 1)
            av_norm(2, 0)
            qk_exp(3, 0)
            av_norm(2, 1)
            qk_exp(3, 1)
            av_norm(3, 0)
            av_norm(3, 1)

            if bname == "xy":
                pending_fc.extend(
                    [("x", 0, ofc) for ofc in range(KC)]
                    + [("x", 1, ofc) for ofc in range(KC)])
            elif bname == "yy":
                # tail: fc(y, qh0) streams while yy qh1's last normalize
                # chains resolve, then fc(y, qh1).
                for ofc in range(KC):
                    fc_chunk("y", 0, ofc)
                for ofc in range(KC):
                    fc_chunk("y", 1, ofc)

    _split_excess_waits(nc)
    return nc


def _get_nc():
    global _CACHED_NC
    if _CACHED_NC is None:
        _CACHED_NC = _build_nc()
    return _CACHED_NC


def kernel(**inputs):
    global LAST_RESULT
    nc = _get_nc()

    X = np.asarray(inputs["X"], np.float32)
    Y = np.asarray(inputs["Y"], np.float32)

    def bf(a):
        return np.ascontiguousarray(a).astype(ml_dtypes.bfloat16)

    shared = {}
    for bn, _, _ in _BRANCHES:
        for t in ("q", "k", "v"):
            shared[f"w{t}_{bn}"] = bf(inputs[f"W_{t}_{bn}"])
            shared[f"b{t}_{bn}"] = np.asarray(inputs[f"b_{t}_{bn}"], np.float32)
    shared["wfc_x"] = bf(inputs["W_X"])
    shared["wfc_y"] = bf(inputs["W_Y"])
    shared["bfc_x"] = np.asarray(inputs["b_X"], np.float32)
    shared["bfc_y"] = np.asarray(inputs["b_Y"], np.float32)

    in_maps = []
    for b in range(B):
        xt = np.ascontiguousarray(X[b].T)
        yt = np.ascontiguousarray(Y[b].T)
        m = dict(shared)
        m["xt_bf"] = xt.astype(ml_dtypes.bfloat16)
        m["yt_bf"] = yt.astype(ml_dtypes.bfloat16)
        m["xt_f32"] = xt
        m["yt_f32"] = yt
        in_maps.append(m)

    res = run_bass_kernel_spmd(nc, in_maps, list(range(B)))
    LAST_RESULT = res

    out_x = np.stack([res.results[b]["out_x_t"].T for b in range(B)])
    out_y = np.stack([res.results[b]["out_y_t"].T for b in range(B)])
    return out_x.astype(np.float32), out_y.astype(np.float32)
